# revision 30
# baseline (speedup 1.0000x reference)
"""Multi-head causal self-attention on 8 trn2 NeuronCores.

Problem: x[4, 2048, 1024], 16 heads of 64 dims, causal softmax attention,
torch-Linear style projections (y = x @ W.T + b).

Sharding: core c = (batch b = c // 2, head-group g = c % 2). Each core
computes the attention output for batch b over heads [8g, 8g+8) and the
partial output projection for those heads' 512 value dims. The host sums
the two head-group partials per batch (the "all-reduce after W_O" of
tensor parallelism, done during unshard) and adds the rank-1 bias
corrections (bv @ Wo.T + bo), which commute with attention because
softmax rows sum to 1.

Numerics: the Q/K projections and the score matmuls run in fp8e4m3 with
perf_mode=DoubleRow (two 128-deep k-tiles per instruction at 0.5
cycles/row): score noise enters the softmax exponent (~1%) and averages
out in P@V, contributing ~2e-3 end-to-end. The V path, P@V, and the
output projection stay fp16 (their error hits the output linearly).
All quantization happens host-side, so weights/activations DMA at 1-2
bytes/elem straight into matmul operands (no on-chip staging).

Device layouts (per core):
  x8   [1024, 2048]  x[b].T in fp8 (Q/K projections)
  x16  [1024, 2048]  x[b].T in fp16 (V projection)
  wq8/wk8 [128, 4, 4, 2, 128] fp8, output-column-permuted (see below)
  wv16 [1024, 512]   Wv[512g:512(g+1), :].T fp16
  wo16 [512, 1024]   Wo.T[512g:512(g+1), :] fp16
  bqp/bkp [512]      bias shards, column-permuted like wq8/wk8
  y    [2048, 1024]  partial output (missing bv/bo rank-1 terms)

Column permutation: PSUM c-chunk p=32*i+l holds dq = 64*H + 32*S + l with
H = 4*(c//2)+i, S = c%2. One [128,512] drain per (c,w) then lands head
H's dh-half S on partitions 32i..32i+32 of a [128, 2, 512] fp8 tile whose
middle dim is the dh-half — exactly the DoubleRow two-k-tile layout the
score matmuls need (contraction 2x32=64 at tile_position row 32i).

On-chip pipeline, interleaved over 512-wide column chunks:
  - Q/K projections: 4 DoubleRow fp8 matmuls per (c,w); V: 8 fp16
    matmuls per (s,w), stored fp16 per head with a ones column so P@V
    also produces the softmax denominators.
  - Scores per head: DoubleRow fp8, k-chunk pairs landing in one 2-bank
    PSUM tile so a single ACT instruction exponentiates both (ACT is
    the kernel's bottleneck engine: ~166us of exp).
  - The causal mask is a multiplicative 0/1 square applied after exp
    (off the scores->exp critical chain, on DVE).
  - P@V' in fp16 with the exp tile stationary, sub-q-outer; PV lags
    behind scores/exp (depth 2 in window 0, 1 in windows 1-2, 0 in the
    last) so it never waits on ACT. 1/denominator folds into the PSUM
    drain (vector engine).
  - Projection/V/tail work is queued as fill steps consumed one per
    score pair, keeping the PE's static instruction order from
    head-of-line blocking ACT behind a long fill burst.
  - Window 3's PV+tail interleave per 128-query sub-chunk so only the
    last sub-chunk's chain trails the final exp.
"""

from collections import deque
from contextlib import ExitStack

import numpy as np

import concourse.bass as bass
import concourse.mybir as mybir
import concourse.tile as tile
from concourse import bacc
from concourse.masks import make_identity

F32 = mybir.dt.float32
F16 = mybir.dt.float16
F8 = mybir.dt.float8e4
F32R = mybir.dt.float32r
Exp = mybir.ActivationFunctionType.Exp
DR = mybir.MatmulPerfMode.DoubleRow
ABL_CORR = True

D = 1024          # model dim
T = 2048          # sequence length
BATCH = 4
NH = 16           # total heads
DH = 64           # head dim
HLOC = 8          # heads per core
DSH = 512         # value dims per core (HLOC * DH)
N_CORES = 8

TC = T // 512     # 4 column tiles of 512
KC = T // 128     # 16 k chunks of 128
DC = D // 128     # 8 contraction chunks for the QKV projections

# PV lag depth per window: how many heads' P@V trail their scores/exp.
PV_DEPTH = (5, 2, 2, 1)


def _col_perm():
    """dq' -> dq permutation for the Q/K projection output columns."""
    perm = np.empty(DSH, dtype=np.int64)
    for c in range(4):
        for i in range(4):
            for l in range(32):
                perm[128 * c + 32 * i + l] = 64 * (4 * (c // 2) + i) + 32 * (c % 2) + l
    return perm


def _build():
    nc = bacc.Bacc("TRN2", target_bir_lowering=False, debug=False,
                   num_devices=N_CORES)
    x8 = nc.dram_tensor("x8", [D, T], F8, kind="ExternalInput").ap()
    x16 = nc.dram_tensor("x16", [D, T], F16, kind="ExternalInput").ap()
    wq8 = nc.dram_tensor("wq8", [128, 4, 4, 2, 128], F8, kind="ExternalInput").ap()
    wk8 = nc.dram_tensor("wk8", [128, 4, 4, 2, 128], F8, kind="ExternalInput").ap()
    x8lo = nc.dram_tensor("x8lo", [D, 128], F8, kind="ExternalInput").ap()
    wq8lo = nc.dram_tensor("wq8lo", [128, 4, 4, 2, 128], F8, kind="ExternalInput").ap()
    wk8lo = nc.dram_tensor("wk8lo", [128, 4, 4, 2, 128], F8, kind="ExternalInput").ap()
    wv16 = nc.dram_tensor("wv16", [D, DSH], F16, kind="ExternalInput").ap()
    wo16 = nc.dram_tensor("wo16", [DSH, D], F16, kind="ExternalInput").ap()
    bqp = nc.dram_tensor("bqp", [DSH], F32, kind="ExternalInput").ap()
    bkp = nc.dram_tensor("bkp", [DSH], F32, kind="ExternalInput").ap()
    y = nc.dram_tensor("y", [T, D], F16, kind="ExternalOutput").ap()

    with tile.TileContext(nc) as tc, ExitStack() as ctx:
        singles = ctx.enter_context(tc.tile_pool(name="singles", bufs=1))
        wpool = ctx.enter_context(tc.tile_pool(name="wpool", bufs=1))
        x8pool = ctx.enter_context(tc.tile_pool(name="x8p", bufs=2))
        x16pool = ctx.enter_context(tc.tile_pool(name="x16p", bufs=2))
        qtpool = ctx.enter_context(tc.tile_pool(name="qt", bufs=2))
        attnp = ctx.enter_context(tc.tile_pool(name="attnp", bufs=3))
        attnTp = ctx.enter_context(tc.tile_pool(name="attnTp", bufs=2))
        exp_pool = ctx.enter_context(tc.tile_pool(name="exp", bufs=24))
        small = ctx.enter_context(tc.tile_pool(name="small", bufs=8))
        ybuf = ctx.enter_context(tc.tile_pool(name="ybuf", bufs=3))
        ps_s = ctx.enter_context(tc.tile_pool(name="ps_s", bufs=2, space="PSUM"))
        ps_pv = ctx.enter_context(tc.tile_pool(name="ps_pv", bufs=2, space="PSUM"))
        ps_fill = ctx.enter_context(tc.tile_pool(name="ps_fill", bufs=2, space="PSUM"))

        # [dk%128, dk//128, 1, t]; the size-1 ktile dim is stride-0
        # broadcast to 2 in the score matmuls: DoubleRow computes
        # K*(Q_hi + Q_lo) at 0.5 cycles/row with Q's fp8 residual in the
        # second rhs tile
        KT_t = singles.tile([128, 4, 1, T], F8)
        Vp_t = singles.tile([128, KC, HLOC, DH + 1], F16)  # [t%128, t//128, h, dv+1]
        ident_t = singles.tile([128, 128], F32)
        mask_t = singles.tile([128, 128], F16)      # 0/1 causal square
        KLO_t = singles.tile([128, 4, 1, 128], F8)   # K residual, tokens 0:128
        bq_t = singles.tile([128, 4], F32)
        bk_t = singles.tile([128, 4], F32)

        make_identity(nc, ident_t)
        nc.vector.memset(Vp_t[:, :, :, DH:DH + 1], 1.0)
        nc.gpsimd.memset(mask_t, 1.0)
        # s_T layout [k, q]: multiplicative 0/1 causal mask for the 128x128
        # diagonal square, applied to exp(s) AFTER the exp (exp(s)*0 ==
        # exp(s-1e6)). Keep 1.0 where (qq - kk) >= 0, else 0.
        nc.gpsimd.affine_select(
            out=mask_t, in_=mask_t,
            compare_op=mybir.AluOpType.is_ge,
            fill=0.0,
            base=0,
            pattern=[[1, 128]],
            channel_multiplier=-1,
        )

        wq8_t = wpool.tile([128, 4, 4, 2, 128], F8)
        wk8_t = wpool.tile([128, 4, 4, 2, 128], F8)
        x8lo_t = wpool.tile([128, DC, 128], F8)
        wq8lo_t = wpool.tile([128, 4, 4, 2, 128], F8)
        wk8lo_t = wpool.tile([128, 4, 4, 2, 128], F8)
        wv16_t = wpool.tile([128, DC, DSH], F16)
        wo16_t = wpool.tile([128, 4, D], F16)
        wv16_r = wv16.rearrange("(d p) j -> p d j", p=128)
        wo16_r = wo16.rearrange("(c p) j -> p c j", p=128)
        x8_r = x8.rearrange("(d p) t -> p d t", p=128)
        x16_r = x16.rearrange("(d p) t -> p d t", p=128)

        # DMA emission order sets queue priority: x8(0), all four Q/K
        # weight column-blocks, biases, then wv16 and x16(0) in 128-token
        # chunks (each V step only reads its own 128 columns, so the
        # first V group unblocks after 1/4 of the x16 bytes land).
        x8t0 = x8pool.tile([128, DC, 512], F8, tag="x8", name="x8t")
        nc.sync.dma_start(out=x8t0[:, 0:4], in_=x8_r[:, 0:4, 0:512])
        nc.sync.dma_start(out=wq8_t[:, 0], in_=wq8[:, 0])
        nc.sync.dma_start(out=wk8_t[:, 0], in_=wk8[:, 0])
        nc.sync.dma_start(out=x8t0[:, 4:8], in_=x8_r[:, 4:8, 0:512])
        nc.sync.dma_start(out=bq_t, in_=bqp.rearrange("(c p) -> p c", p=128))
        nc.sync.dma_start(out=bk_t, in_=bkp.rearrange("(c p) -> p c", p=128))
        nc.sync.dma_start(out=x8lo_t, in_=x8lo.rearrange("(d p) t -> p d t", p=128))
        nc.sync.dma_start(out=wq8lo_t[:, 0], in_=wq8lo[:, 0])
        nc.sync.dma_start(out=wk8lo_t[:, 0], in_=wk8lo[:, 0])
        nc.sync.dma_start(out=wq8lo_t[:, 1:4], in_=wq8lo[:, 1:4])
        nc.sync.dma_start(out=wk8lo_t[:, 1:4], in_=wk8lo[:, 1:4])
        nc.sync.dma_start(out=wq8_t[:, 1:4], in_=wq8[:, 1:4])
        nc.sync.dma_start(out=wk8_t[:, 1:4], in_=wk8[:, 1:4])
        nc.sync.dma_start(out=wv16_t, in_=wv16_r)
        x16t0 = x16pool.tile([128, DC, 512], F16, tag="x16", name="x16t")
        for s2 in range(2):
            nc.sync.dma_start(out=x16t0[:, :, 256 * s2:256 * (s2 + 1)],
                              in_=x16_r[:, :, 256 * s2:256 * (s2 + 1)])

        from collections import defaultdict
        by_key = defaultdict(list)
        fills = deque()   # step dicts consumed by budget-paced feed()

        def step(weight, key, fn):
            st = {"wt": weight, "fn": fn, "done": False}
            by_key[key].append(st)
            return st

        def run_step(st):
            if not st["done"]:
                st["done"] = True
                st["fn"]()

        def force(key):
            for st in by_key.get(key, ()):
                run_step(st)

        def proj_steps(w, box):
            """Weighted fill steps for window w: x/qt alloc + Q/K groups
            (DoubleRow fp8, deadline-keyed per c-chunk) and V halves
            (fp16, 256 dv wide so no step exceeds ~900ns of PE time)."""

            def alloc(w=w):
                if w == 0:
                    box["x8"], box["x16"] = x8t0, x16t0
                else:
                    x8t = x8pool.tile([128, DC, 512], F8, tag="x8", name="x8t")
                    nc.sync.dma_start(out=x8t, in_=x8_r[:, :, 512 * w:512 * (w + 1)])
                    x16t = x16pool.tile([128, DC, 512], F16, tag="x16", name="x16t")
                    for s2 in range(2):
                        nc.sync.dma_start(
                            out=x16t[:, :, 256 * s2:256 * (s2 + 1)],
                            in_=x16_r[:, :, 512 * w + 256 * s2:512 * w + 256 * (s2 + 1)])
                    box["x8"], box["x16"] = x8t, x16t
                box["qt"] = qtpool.tile([128, 4, 2, 512], F8, tag="qt",
                                        name="qt_w")

            def qkstep(c, wt, wlo, dst_f, w=w):
                x8t = box["x8"]
                psp = ps_fill.tile([128, 512], F32, tag="fill", name="psqk")
                # base matmuls over full 512 tokens; for window 0 add fp8
                # residual cross terms (x_lo*w_hi + x_hi*w_lo) on the first
                # 128 tokens: causally-peaked softmax rows live there and
                # amplify score noise; elsewhere fp8 noise averages out.
                mms = [(psp, wt[:, c, jp], x8t[:, 2 * jp:2 * jp + 2, :])
                       for jp in range(4)]
                if w == 0:
                    for jp in range(4):
                        mms.append((psp[:, 0:128], wlo[:, c, jp],
                                    x8t[:, 2 * jp:2 * jp + 2, 0:128]))
                        mms.append((psp[:, 0:128], wt[:, c, jp],
                                    x8lo_t[:, 2 * jp:2 * jp + 2, :]))
                for n, (out, lhsT, rhs) in enumerate(mms):
                    nc.tensor.matmul(
                        out, lhsT=lhsT, rhs=rhs,
                        start=(n == 0), stop=(n == len(mms) - 1),
                        perf_mode=DR, skip_group_check=True,
                    )
                dst_f(c, psp)

            def qdrain(c, psp):
                qt_w = box["qt"]
                nc.vector.tensor_scalar_add(qt_w[:, c, 0, :], psp,
                                            bq_t[:, c:c + 1])
                # ktile1 = fp8 residual (Q - fp8(Q)): DoubleRow then scores
                # K*(Q_hi + Q_lo), cancelling the drain quantization
                nc.vector.tensor_tensor(
                    out=qt_w[:, c, 1, :], in0=psp, in1=qt_w[:, c, 0, :],
                    op=mybir.AluOpType.subtract)

            def kdrain(c, psp, w=w):
                nc.vector.tensor_scalar_add(
                    KT_t[:, c, 0, 512 * w:512 * (w + 1)], psp,
                    bk_t[:, c:c + 1])
                if w == 0:
                    # fp8 K residual for the first diagonal square
                    nc.vector.tensor_tensor(
                        out=KLO_t[:, c, 0, :], in0=psp[:, 0:128],
                        in1=KT_t[:, c, 0, 0:128],
                        op=mybir.AluOpType.subtract)

            def vhalf(s, hf, w=w):
                x16t = box["x16"]
                psv = ps_fill.tile([128, 256], F32, tag="fill", name="psv")
                for d in range(DC):
                    nc.tensor.matmul(
                        psv,
                        lhsT=x16t[:, d, 128 * s:128 * (s + 1)],
                        rhs=wv16_t[:, d, 256 * hf:256 * (hf + 1)],
                        start=(d == 0), stop=(d == DC - 1),
                    )
                nc.vector.tensor_copy(
                    Vp_t[:, 4 * w + s, 4 * hf:4 * (hf + 1), 0:DH],
                    psv.rearrange("p (h v) -> p h v", h=4),
                )

            qk = {}
            qk[0] = [step(550, ("qk", w, 0),
                          lambda: qkstep(0, wq8_t, wq8lo_t, qdrain)),
                     step(550, ("qk", w, 0),
                          lambda: qkstep(0, wk8_t, wk8lo_t, kdrain))]
            for c in range(1, 4):
                qk[c] = [step(550, ("qk", w, c),
                              lambda c=c: qkstep(c, wq8_t, wq8lo_t, qdrain)),
                         step(550, ("qk", w, c),
                              lambda c=c: qkstep(c, wk8_t, wk8lo_t, kdrain))]
            vs = [step(900, ("v", w, s), lambda s=s, hf=hf: vhalf(s, hf))
                  for s in range(4) for hf in range(2)]
            # interleave: c-group deadlines are heads 2c, V(w) is needed by
            # the first P@V pop of window w. The alloc step (x DMA issue)
            # is returned separately -- the driver runs it immediately at
            # the previous window's start so the transfers land in time.
    

            return (step(0, ("qk", w, 0), alloc),
                    qk[0] + qk[1] + vs[0:2] + qk[2] + vs[2:4]
                    + qk[3] + vs[4:8])

        def emit_scores_exp(w, h, qt, feed):
            """DoubleRow fp8 scores + paired exp for head h of window w.
            Calls feed() after each pair (fill-step pacing). Returns the
            list of (exp_tile, sub) chunk handles."""
            kmax = 4 * (w + 1)
            ch, po = h // 2, (h % 2) * 64
            ex_buf = []
            for jp in range(kmax // 2):
                pssb = ps_s.tile([128, 2, 512], F32, tag="pss", name="pss")
                exb = exp_pool.tile([128, 2, 512], F16, tag="ex", name="ex")
                rel0 = 2 * jp - 4 * w
                # both matmuls write from the PAIR's first live column (the
                # second diag chunk's extra 128 columns are causally dead but
                # keep the paired exp's input region initialized)
                q0 = max(rel0, 0) * 128
                for sub in range(2):
                    j = 2 * jp + sub
                    corr = (w == 0 and j == 0) and ABL_CORR
                    nc.tensor.matmul(
                        pssb[:, sub, q0:],
                        lhsT=KT_t[po:po + 64, ch, :,
                                  128 * j:128 * (j + 1)].broadcast_to(
                                      [64, 2, 128]),
                        rhs=qt[po:po + 64, ch, :, q0:],
                        start=True, stop=not corr,
                        perf_mode=DR, skip_group_check=True,
                    )
                    if corr:
                        # += K_lo * (Q_hi + Q_lo) on the causally-peaked
                        # first square (k<128, q<128)
                        nc.tensor.matmul(
                            pssb[:, sub, 0:128],
                            lhsT=KLO_t[po:po + 64, ch, :, :].broadcast_to(
                                [64, 2, 128]),
                            rhs=qt[po:po + 64, ch, :, 0:128],
                            start=False, stop=True,
                            perf_mode=DR, skip_group_check=True,
                        )
                nc.scalar.activation(out=exb[:, :, q0:],
                                     in_=pssb[:, :, q0:],
                                     func=Exp, scale=0.125 / 1024.0)
                for sub in range(2):
                    rel = 2 * jp + sub - 4 * w
                    if rel >= 0:
                        qq = rel * 128
                        # zero exp(s) above the diagonal; only PV of
                        # sub-q i == rel reads this square
                        nc.gpsimd.tensor_mul(
                            exb[:, sub, qq:qq + 128],
                            exb[:, sub, qq:qq + 128], mask_t)
                ex_buf.append((exb, 0))
                ex_buf.append((exb, 1))
                # pair's ACT time minus its PE time funds the fill budget
                feed((2 * (512 - q0)) * 0.833 + 185.0
                     - (2 * (512 - q0)) * 0.417)
            return ex_buf

        def emit_pv_sub(w, h, ex_buf, attn_t, i):
            """P@V' + rescale for one 128-query sub-chunk."""
            pso = ps_pv.tile([128, DH + 1], F32, tag="pso", name="pso")
            jlast = 4 * w + i
            for j in range(jlast + 1):
                exb, sub = ex_buf[j]
                nc.tensor.matmul(
                    pso,
                    lhsT=exb[:, sub, 128 * i:128 * (i + 1)],
                    rhs=Vp_t[:, j, h, :],
                    start=(j == 0), stop=(j == jlast),
                )
            rec = small.tile([128, 1], F32, tag="rec", name="rec")
            nc.vector.reciprocal(rec, pso[:, DH:DH + 1])
            nc.vector.tensor_mul(
                attn_t[:, i, DH * h:DH * (h + 1)],
                pso[:, 0:DH],
                rec.broadcast_to([128, DH]),
            )

        def emit_pv(w, h, ex_buf, attn_t):
            for i in range(4):
                emit_pv_sub(w, h, ex_buf, attn_t, i)

        def tail_sub(w, attn_t, i, last=False):
            """Transpose + W_O + store for one 128-query sub-chunk."""
            drain = nc.scalar.copy if last else nc.vector.tensor_copy
            atT = attnTp.tile([128, 4, 128], F16, tag="attnT", name="attnT")
            pst = ps_fill.tile([128, 512], F32, tag="fill", name="pst")
            for c in range(4):
                nc.tensor.transpose(
                    pst[:, 128 * c:128 * (c + 1)],
                    attn_t[:, i, 128 * c:128 * (c + 1)], ident_t)
            drain(atT, pst.rearrange("p (c q) -> p c q", c=4))
            ysb = ybuf.tile([128, 2, 512], F16, tag="ysb", name="ysb")
            for jc in range(2):
                py = ps_fill.tile([128, 512], F32, tag="fill", name="py")
                for c in range(4):
                    nc.tensor.matmul(
                        py,
                        lhsT=atT[:, c, :],
                        rhs=wo16_t[:, c, 512 * jc:512 * (jc + 1)],
                        start=(c == 0), stop=(c == 3),
                    )
                drain(ysb[:, jc, :], py)
            eng = nc.sync if last else nc.gpsimd
            eng.dma_start(
                out=y[512 * w + 128 * i:512 * w + 128 * (i + 1), :],
                in_=ysb,
            )

        # ---- driver ----
        pend = deque()    # (w, h, ex_buf, attn_t) awaiting P@V
        prio = deque()    # lazy P@V sub-chunk steps, drained before fills
        budget = [0.0]    # fill-step PE budget (ns), fed per score pair

        def feed(gain=611.0):
            budget[0] = min(budget[0] + gain, 1500.0)
            for q in (prio, fills):
                while q:
                    st = q[0]
                    if st["done"]:
                        q.popleft()
                        continue
                    if st["wt"] <= budget[0]:
                        budget[0] -= st["wt"]
                        q.popleft()
                        run_step(st)
                    else:
                        return

        def force_pv_all(pw):
            for ph in range(HLOC):
                force(("pv", pw, ph))

        def pv_sub_forced(pw, ph, pex, pat, i):
            for s in range(i + 1):
                force(("v", pw, s))
            emit_pv_sub(pw, ph, pex, pat, i)

        def enqueue_pv(pw, ph, pex, pat):
            """Queue P@V for (pw, ph) as 4 small prio steps (one per
            128-query sub-chunk) so feed() interleaves them into the
            ACT-paced score-pair stream."""
            if pw >= 2:
                force(("tail", pw - 2))   # attn ring (bufs=3) safety
            for i in range(4):
                prio.append(step(
                    (4 * pw + i + 1) * 27 + 60, ("pv", pw, ph),
                    lambda i=i: pv_sub_forced(pw, ph, pex, pat, i)))
            if ph == HLOC - 1:     # window pw fully rescaled -> tail it
                force(("wo",))
                for i in range(4):
                    def tfn(pw=pw, pat=pat, i=i):
                        force_pv_all(pw)
                        tail_sub(pw, pat, i)
                    fills.append(step(900, ("tail", pw), tfn))

        box0 = {}
        a0, s0 = proj_steps(0, box0)
        run_step(a0)
        force(("qk", 0, 0))        # Q/K c0: first scores input
        qt_cur = box0["qt"]
        fills.extend(s0)           # remaining w0 steps (done ones skipped)
        box = {}
        a1, nxt = proj_steps(1, box)
        run_step(a1)               # x8/x16(1) DMAs queue behind w0 loads
        fills.extend(nxt)
        wo_st = step(0, ("wo",),
                     lambda: nc.sync.dma_start(out=wo16_t, in_=wo16_r))
        run_step(wo_st)

        for w in range(TC):
            if 1 <= w < TC - 1:
                a2, s2 = proj_steps(w + 1, box)
                run_step(a2)       # issue x8/x16(w+1) DMAs now
                fills.extend(s2)
            if w >= 3:
                force(("tail", w - 3))   # attn ring slot reuse (bufs=3)
            attn_t = attnp.tile([128, 4, DSH], F32, tag="attn", name="attn_t")
            depth = PV_DEPTH[w]
            for h in range(HLOC):
                force(("qk", w, h // 2))
                last_head = (w == TC - 1 and h == HLOC - 1)
                if last_head:
                    # hand pending P@V to the prio queue so it overlaps
                    # this head's pairs
                    while pend:
                        pw, ph, pex, pat = pend.popleft()
                        enqueue_pv(pw, ph, pex, pat)
                ex = emit_scores_exp(w, h, qt_cur, feed)
                if last_head:
                    for st in list(prio):
                        run_step(st)
                    prio.clear()
                    for s in range(4):
                        force(("v", w, s))
                    force(("wo",))
                    # interleave PV+tail per sub-chunk; sub-q 2/3 both wait
                    # on the final exp pair, so emit their PVs together and
                    # pipeline the two tails
                    for i in range(2):
                        emit_pv_sub(w, h, ex, attn_t, i)
                        tail_sub(w, attn_t, i, last=True)
                    emit_pv_sub(w, h, ex, attn_t, 2)
                    emit_pv_sub(w, h, ex, attn_t, 3)
                    tail_sub(w, attn_t, 2, last=True)
                    tail_sub(w, attn_t, 3, last=True)
                    continue
                pend.append((w, h, ex, attn_t))
                npop = 0
                while len(pend) > depth and npop < 2:
                    npop += 1
                    pw, ph, pex, pat = pend.popleft()
                    enqueue_pv(pw, ph, pex, pat)
            if w + 1 < TC:
                force(("qk", w + 1, 0))  # next window's first scores input
                qt_cur = box["qt"]
                box = {}
        for st in list(prio) + list(fills):
            run_step(st)
    nc.compile()
    return nc


def shard_inputs(x, Wq, bq, Wk, bk, Wv, bv, Wo, bo):
    """Returns the 8 per-core input maps (host-side quantization)."""
    import ml_dtypes
    f8 = ml_dtypes.float8_e4m3fn
    # fp8e4m3's normal range floor is 2^-6: scale x by 4 and W by 8 so
    # weights (sigma 0.02) and their residuals escape the subnormal zone
    # while scaled scores (<=32k) stay inside fp16 range on the ACT input
    # path. The exp scale divides the 32^2 back out.
    XS, WS = 4.0, 8.0
    in_maps = []
    for c in range(N_CORES):
        b, g = c // 2, c % 2
        sl = slice(DSH * g, DSH * (g + 1))
        xT = np.ascontiguousarray(x[b].T)

        def pack_qk(W):
            a = W[sl].T.reshape(4, 2, 128, 4, 128)  # [jp, i, p, c, q]
            return (WS * np.ascontiguousarray(
                a.transpose(2, 3, 0, 1, 4))).astype(f8)

        x8_hi = (XS * xT).astype(f8)
        wq_hi = pack_qk(Wq)
        wk_hi = pack_qk(Wk)

        def pack_qk_lo(W, hi):
            a = W[sl].T.reshape(4, 2, 128, 4, 128)
            a = np.ascontiguousarray(a.transpose(2, 3, 0, 1, 4))
            return (WS * a - hi.astype(np.float32)).astype(f8)

        in_maps.append({
            "x8": x8_hi,
            "x8lo": (XS * xT[:, 0:128]
                     - x8_hi[:, 0:128].astype(np.float32)).astype(f8),
            "wq8lo": pack_qk_lo(Wq, wq_hi),
            "wk8lo": pack_qk_lo(Wk, wk_hi),
            "x16": xT.astype(np.float16),
            "wq8": wq_hi,
            "wk8": wk_hi,
            "wv16": np.ascontiguousarray(Wv[sl].T).astype(np.float16),
            "wo16": np.ascontiguousarray(Wo.T[sl]).astype(np.float16),
            "bqp": (XS * WS * np.ascontiguousarray(bq[sl])).astype(np.float32),
            "bkp": (XS * WS * np.ascontiguousarray(bk[sl])).astype(np.float32),
        })
    return in_maps


def combine_outputs(results, bv, Wo, bo):
    """Sum head-group partials per batch + rank-1 bias corrections."""
    corr = (bv @ Wo.T + bo).astype(np.float32)  # [D]; exact because softmax
    y = np.empty((BATCH, T, D), dtype=np.float32)  # rows sum to 1
    for b in range(BATCH):
        y[b] = (results[2 * b]["y"].astype(np.float32)
                + results[2 * b + 1]["y"].astype(np.float32) + corr)
    return y


def run_sharded(inputs, trace=False):
    """Build, compile, run on cores 0-7. Returns (y_full, BassKernelResults)."""
    from concourse import bass_utils

    inputs = {k: np.asarray(v, dtype=np.float32) for k, v in inputs.items()}
    nc = _build()
    in_maps = shard_inputs(
        inputs["x"], inputs["Wq"], inputs["bq"], inputs["Wk"], inputs["bk"],
        inputs["Wv"], inputs["bv"], inputs["Wo"], inputs["bo"])
    res = bass_utils.run_bass_kernel_spmd(
        nc, in_maps, list(range(N_CORES)), trace=trace)
    y = combine_outputs(res.results, inputs["bv"], inputs["Wo"], inputs["bo"])
    return y, res


def kernel(**inputs):
    y, _ = run_sharded(inputs, trace=False)
    return y


if __name__ == "__main__":
    rng = np.random.default_rng(0)
    demo = {
        "x": rng.standard_normal((BATCH, T, D), dtype=np.float32),
        "Wq": rng.standard_normal((D, D), dtype=np.float32) * 0.02,
        "bq": np.zeros(D, np.float32),
        "Wk": rng.standard_normal((D, D), dtype=np.float32) * 0.02,
        "bk": np.zeros(D, np.float32),
        "Wv": rng.standard_normal((D, D), dtype=np.float32) * 0.02,
        "bv": np.zeros(D, np.float32),
        "Wo": rng.standard_normal((D, D), dtype=np.float32) * 0.02,
        "bo": np.zeros(D, np.float32),
    }
    out = kernel(**demo)
    print(out.shape, out.dtype)


# revision 34
# speedup vs baseline: 1.0264x; 1.0264x over previous
"""Multi-head causal self-attention on 8 trn2 NeuronCores.

Problem: x[4, 2048, 1024], 16 heads of 64 dims, causal softmax attention,
torch-Linear style projections (y = x @ W.T + b).

Sharding: core c = (batch b = c // 2, head-group g = c % 2). Each core
computes the attention output for batch b over heads [8g, 8g+8) and the
partial output projection for those heads' 512 value dims. The host sums
the two head-group partials per batch (the "all-reduce after W_O" of
tensor parallelism, done during unshard) and adds the rank-1 bias
corrections (bv @ Wo.T + bo), which commute with attention because
softmax rows sum to 1.

Numerics: the Q/K projections and the score matmuls run in fp8e4m3 with
perf_mode=DoubleRow (two 128-deep k-tiles per instruction at 0.5
cycles/row): score noise enters the softmax exponent (~1%) and averages
out in P@V, contributing ~2e-3 end-to-end. The V path, P@V, and the
output projection stay fp16 (their error hits the output linearly).
All quantization happens host-side, so weights/activations DMA at 1-2
bytes/elem straight into matmul operands (no on-chip staging).

Device layouts (per core):
  x8   [1024, 2048]  x[b].T in fp8 (Q/K projections)
  x16  [1024, 2048]  x[b].T in fp16 (V projection)
  wq8/wk8 [128, 4, 4, 2, 128] fp8, output-column-permuted (see below)
  wv16 [1024, 512]   Wv[512g:512(g+1), :].T fp16
  wo16 [512, 1024]   Wo.T[512g:512(g+1), :] fp16
  bqp/bkp [512]      bias shards, column-permuted like wq8/wk8
  y    [2048, 1024]  partial output (missing bv/bo rank-1 terms)

Column permutation: PSUM c-chunk p=32*i+l holds dq = 64*H + 32*S + l with
H = 4*(c//2)+i, S = c%2. One [128,512] drain per (c,w) then lands head
H's dh-half S on partitions 32i..32i+32 of a [128, 2, 512] fp8 tile whose
middle dim is the dh-half — exactly the DoubleRow two-k-tile layout the
score matmuls need (contraction 2x32=64 at tile_position row 32i).

On-chip pipeline, interleaved over 512-wide column chunks:
  - Q/K projections: 4 DoubleRow fp8 matmuls per (c,w); V: 8 fp16
    matmuls per (s,w), stored fp16 per head with a ones column so P@V
    also produces the softmax denominators.
  - Scores per head: DoubleRow fp8, k-chunk pairs landing in one 2-bank
    PSUM tile so a single ACT instruction exponentiates both (ACT is
    the kernel's bottleneck engine: ~166us of exp).
  - The causal mask is a multiplicative 0/1 square applied after exp
    (off the scores->exp critical chain, on DVE).
  - P@V' in fp16 with the exp tile stationary, sub-q-outer; PV lags
    behind scores/exp (depth 2 in window 0, 1 in windows 1-2, 0 in the
    last) so it never waits on ACT. 1/denominator folds into the PSUM
    drain (vector engine).
  - Projection/V/tail work is queued as fill steps consumed one per
    score pair, keeping the PE's static instruction order from
    head-of-line blocking ACT behind a long fill burst.
  - Window 3's PV+tail interleave per 128-query sub-chunk so only the
    last sub-chunk's chain trails the final exp.
"""

from collections import deque
from contextlib import ExitStack

import numpy as np

import concourse.bass as bass
import concourse.mybir as mybir
import concourse.tile as tile
from concourse import bacc
from concourse.masks import make_identity

F32 = mybir.dt.float32
F16 = mybir.dt.float16
F8 = mybir.dt.float8e4
F32R = mybir.dt.float32r
Exp = mybir.ActivationFunctionType.Exp
DR = mybir.MatmulPerfMode.DoubleRow
ABL_CORR = True

D = 1024          # model dim
T = 2048          # sequence length
BATCH = 4
NH = 16           # total heads
DH = 64           # head dim
HLOC = 8          # heads per core
DSH = 512         # value dims per core (HLOC * DH)
N_CORES = 8

TC = T // 512     # 4 column tiles of 512
KC = T // 128     # 16 k chunks of 128
DC = D // 128     # 8 contraction chunks for the QKV projections

# PV lag depth per window: how many heads' P@V trail their scores/exp.
PV_DEPTH = (5, 2, 2, 1)


def _col_perm():
    """dq' -> dq permutation for the Q/K projection output columns."""
    perm = np.empty(DSH, dtype=np.int64)
    for c in range(4):
        for i in range(4):
            for l in range(32):
                perm[128 * c + 32 * i + l] = 64 * (4 * (c // 2) + i) + 32 * (c % 2) + l
    return perm


def _build():
    nc = bacc.Bacc("TRN2", target_bir_lowering=False, debug=False,
                   num_devices=N_CORES)
    x8 = nc.dram_tensor("x8", [D, T], F8, kind="ExternalInput").ap()
    wq8 = nc.dram_tensor("wq8", [128, 4, 4, 2, 128], F8, kind="ExternalInput").ap()
    wk8 = nc.dram_tensor("wk8", [128, 4, 4, 2, 128], F8, kind="ExternalInput").ap()
    x8lo = nc.dram_tensor("x8lo", [D, 128], F8, kind="ExternalInput").ap()
    wq8lo = nc.dram_tensor("wq8lo", [128, 4, 4, 2, 128], F8, kind="ExternalInput").ap()
    wk8lo = nc.dram_tensor("wk8lo", [128, 4, 4, 2, 128], F8, kind="ExternalInput").ap()
    wv8 = nc.dram_tensor("wv8", [128, 4, 2, DSH], F8, kind="ExternalInput").ap()
    wv8lo = nc.dram_tensor("wv8lo", [128, 4, 2, DSH], F8, kind="ExternalInput").ap()
    wo16 = nc.dram_tensor("wo16", [DSH, D], F16, kind="ExternalInput").ap()
    bqp = nc.dram_tensor("bqp", [DSH], F32, kind="ExternalInput").ap()
    bkp = nc.dram_tensor("bkp", [DSH], F32, kind="ExternalInput").ap()
    y = nc.dram_tensor("y", [T, D], F16, kind="ExternalOutput").ap()

    with tile.TileContext(nc) as tc, ExitStack() as ctx:
        singles = ctx.enter_context(tc.tile_pool(name="singles", bufs=1))
        wpool = ctx.enter_context(tc.tile_pool(name="wpool", bufs=1))
        x8pool = ctx.enter_context(tc.tile_pool(name="x8p", bufs=2))
        qtpool = ctx.enter_context(tc.tile_pool(name="qt", bufs=2))
        attnp = ctx.enter_context(tc.tile_pool(name="attnp", bufs=3))
        attnTp = ctx.enter_context(tc.tile_pool(name="attnTp", bufs=2))
        exp_pool = ctx.enter_context(tc.tile_pool(name="exp", bufs=24))
        small = ctx.enter_context(tc.tile_pool(name="small", bufs=8))
        ybuf = ctx.enter_context(tc.tile_pool(name="ybuf", bufs=3))
        ps_s = ctx.enter_context(tc.tile_pool(name="ps_s", bufs=2, space="PSUM"))
        ps_pv = ctx.enter_context(tc.tile_pool(name="ps_pv", bufs=2, space="PSUM"))
        ps_fill = ctx.enter_context(tc.tile_pool(name="ps_fill", bufs=2, space="PSUM"))

        # [dk%128, dk//128, 1, t]; the size-1 ktile dim is stride-0
        # broadcast to 2 in the score matmuls: DoubleRow computes
        # K*(Q_hi + Q_lo) at 0.5 cycles/row with Q's fp8 residual in the
        # second rhs tile
        KT_t = singles.tile([128, 4, 1, T], F8)
        Vp_t = singles.tile([128, KC, HLOC, DH + 1], F16)  # [t%128, t//128, h, dv+1]
        ident_t = singles.tile([128, 128], F32)
        mask_t = singles.tile([128, 128], F16)      # 0/1 causal square
        KLO_t = singles.tile([128, 4, 1, 128], F8)   # K residual, tokens 0:128
        bq_t = singles.tile([128, 4], F32)
        bk_t = singles.tile([128, 4], F32)

        make_identity(nc, ident_t)
        nc.vector.memset(Vp_t[:, :, :, DH:DH + 1], float(4.0 * 8.0))
        nc.gpsimd.memset(mask_t, 1.0)
        # s_T layout [k, q]: multiplicative 0/1 causal mask for the 128x128
        # diagonal square, applied to exp(s) AFTER the exp (exp(s)*0 ==
        # exp(s-1e6)). Keep 1.0 where (qq - kk) >= 0, else 0.
        nc.gpsimd.affine_select(
            out=mask_t, in_=mask_t,
            compare_op=mybir.AluOpType.is_ge,
            fill=0.0,
            base=0,
            pattern=[[1, 128]],
            channel_multiplier=-1,
        )

        wq8_t = wpool.tile([128, 4, 4, 2, 128], F8)
        wk8_t = wpool.tile([128, 4, 4, 2, 128], F8)
        x8lo_t = wpool.tile([128, DC, 128], F8)
        wq8lo_t = wpool.tile([128, 4, 4, 2, 128], F8)
        wk8lo_t = wpool.tile([128, 4, 4, 2, 128], F8)
        wv8_t = wpool.tile([128, 4, 2, DSH], F8)
        wv8lo_t = wpool.tile([128, 4, 2, DSH], F8)
        wo16_t = wpool.tile([128, 4, D], F16)
        wo16_r = wo16.rearrange("(c p) j -> p c j", p=128)
        x8_r = x8.rearrange("(d p) t -> p d t", p=128)

        # DMA emission order sets queue priority: x8(0), all four Q/K
        # weight column-blocks, biases, then wv16 and x16(0) in 128-token
        # chunks (each V step only reads its own 128 columns, so the
        # first V group unblocks after 1/4 of the x16 bytes land).
        x8t0 = x8pool.tile([128, DC, 512], F8, tag="x8", name="x8t")
        nc.sync.dma_start(out=x8t0, in_=x8_r[:, :, 0:512])
        nc.sync.dma_start(out=wq8_t[:, 0], in_=wq8[:, 0])
        nc.sync.dma_start(out=wk8_t[:, 0], in_=wk8[:, 0])
        nc.sync.dma_start(out=bq_t, in_=bqp.rearrange("(c p) -> p c", p=128))
        nc.sync.dma_start(out=bk_t, in_=bkp.rearrange("(c p) -> p c", p=128))
        nc.sync.dma_start(out=x8lo_t, in_=x8lo.rearrange("(d p) t -> p d t", p=128))
        nc.sync.dma_start(out=wq8lo_t[:, 0], in_=wq8lo[:, 0])
        nc.sync.dma_start(out=wk8lo_t[:, 0], in_=wk8lo[:, 0])
        nc.sync.dma_start(out=wq8lo_t[:, 1:4], in_=wq8lo[:, 1:4])
        nc.sync.dma_start(out=wk8lo_t[:, 1:4], in_=wk8lo[:, 1:4])
        nc.sync.dma_start(out=wq8_t[:, 1:4], in_=wq8[:, 1:4])
        nc.sync.dma_start(out=wk8_t[:, 1:4], in_=wk8[:, 1:4])
        nc.sync.dma_start(out=wv8_t, in_=wv8)
        nc.sync.dma_start(out=wv8lo_t, in_=wv8lo)

        from collections import defaultdict
        by_key = defaultdict(list)
        fills = deque()   # step dicts consumed by budget-paced feed()

        def step(weight, key, fn):
            st = {"wt": weight, "fn": fn, "done": False}
            by_key[key].append(st)
            return st

        def run_step(st):
            if not st["done"]:
                st["done"] = True
                st["fn"]()

        def force(key):
            for st in by_key.get(key, ()):
                run_step(st)

        def proj_steps(w, box):
            """Weighted fill steps for window w: x/qt alloc + Q/K groups
            (DoubleRow fp8, deadline-keyed per c-chunk) and V halves
            (fp16, 256 dv wide so no step exceeds ~900ns of PE time)."""

            def alloc(w=w):
                if w == 0:
                    box["x8"] = x8t0
                else:
                    x8t = x8pool.tile([128, DC, 512], F8, tag="x8", name="x8t")
                    nc.sync.dma_start(out=x8t, in_=x8_r[:, :, 512 * w:512 * (w + 1)])
                    box["x8"] = x8t
                box["qt"] = qtpool.tile([128, 4, 2, 512], F8, tag="qt",
                                        name="qt_w")

            def qkstep(c, wt, dst_f, w=w):
                x8t = box["x8"]
                psp = ps_fill.tile([128, 512], F32, tag="fill", name="psqk")
                for jp in range(4):
                    nc.tensor.matmul(
                        psp, lhsT=wt[:, c, jp],
                        rhs=x8t[:, 2 * jp:2 * jp + 2, :],
                        start=(jp == 0), stop=(jp == 3),
                        perf_mode=DR,
                    )
                dst_f(c, psp)
                return psp

            def qkcross(c, wt, wlo, psp, dst2_f, w=w):
                # fp8 residual cross terms (x_lo*w_hi + x_hi*w_lo) patched
                # onto tokens 0:128 (the causally-peaked softmax rows) as a
                # second accumulation pass + drain; everything outside the
                # first diagonal square never reads the patched columns.
                x8t = box["x8"]
                for jp in range(4):
                    nc.tensor.matmul(
                        psp[:, 0:128], lhsT=wlo[:, c, jp],
                        rhs=x8t[:, 2 * jp:2 * jp + 2, 0:128],
                        start=False, stop=False,
                        perf_mode=DR, skip_group_check=True,
                    )
                    nc.tensor.matmul(
                        psp[:, 0:128], lhsT=wt[:, c, jp],
                        rhs=x8lo_t[:, 2 * jp:2 * jp + 2, :],
                        start=False, stop=(jp == 3),
                        perf_mode=DR, skip_group_check=True,
                    )
                dst2_f(c, psp)

            def qdrain(c, psp, cols=slice(None), w=w):
                qt_w = box["qt"]
                nc.vector.tensor_scalar_add(qt_w[:, c, 0, cols], psp[:, cols],
                                            bq_t[:, c:c + 1])
                # ktile1 = fp8 residual (Q - fp8(Q)): DoubleRow then scores
                # K*(Q_hi + Q_lo), cancelling the drain quantization. For
                # the startup-critical first drain, emit the half the first
                # diagonal pair reads before the rest.
                spans = ((slice(256, 512), slice(0, 256))
                         if (w == 0 and c == 0 and cols == slice(None))
                         else (cols,))
                for cc in spans:
                    nc.vector.tensor_tensor(
                        out=qt_w[:, c, 1, cc], in0=psp[:, cc],
                        in1=qt_w[:, c, 0, cc],
                        op=mybir.AluOpType.subtract)

            def qdrain2(c, psp):
                qdrain(c, psp, cols=slice(0, 128))

            def kdrain(c, psp, w=w):
                nc.vector.tensor_scalar_add(
                    KT_t[:, c, 0, 512 * w:512 * (w + 1)], psp,
                    bk_t[:, c:c + 1])

            def kdrain2(c, psp, w=w):
                nc.vector.tensor_scalar_add(
                    KT_t[:, c, 0, 0:128], psp[:, 0:128], bk_t[:, c:c + 1])
                # fp8 K residual for the first diagonal square
                nc.vector.tensor_tensor(
                    out=KLO_t[:, c, 0, :], in0=psp[:, 0:128],
                    in1=KT_t[:, c, 0, 0:128],
                    op=mybir.AluOpType.subtract)

            def vstep(s, w=w):
                x8t = box["x8"]
                psv = ps_fill.tile([128, DSH], F32, tag="fill", name="psv")
                mms = [(x8t[:, 2 * jp:2 * jp + 2, 128 * s:128 * (s + 1)],
                        wv8_t[:, jp]) for jp in range(4)]
                if w == 0 and s == 0:
                    # residual cross terms for the causally-peaked tokens
                    for jp in range(4):
                        mms.append((x8lo_t[:, 2 * jp:2 * jp + 2, :],
                                    wv8_t[:, jp]))
                        mms.append((x8t[:, 2 * jp:2 * jp + 2, 0:128],
                                    wv8lo_t[:, jp]))
                for n, (lhsT, rhs) in enumerate(mms):
                    nc.tensor.matmul(
                        psv, lhsT=lhsT, rhs=rhs,
                        start=(n == 0), stop=(n == len(mms) - 1),
                        perf_mode=DR, skip_group_check=True,
                    )
                nc.vector.tensor_copy(
                    Vp_t[:, 4 * w + s, :, 0:DH],
                    psv.rearrange("p (h v) -> p h v", h=HLOC),
                )

            def qk_pair(c, wt, wlo, d1, d2, w=w):
                psp = qkstep(c, wt, d1)
                if w == 0:
                    step(300, ("qk2", 0, c),
                         lambda: qkcross(c, wt, wlo, psp, d2))

            qk = {}
            for c in range(4):
                qk[c] = [step(550, ("qk", w, c),
                              lambda c=c: qk_pair(c, wq8_t, wq8lo_t,
                                                  qdrain, qdrain2)),
                         step(550, ("qk", w, c),
                              lambda c=c: qk_pair(c, wk8_t, wk8lo_t,
                                                  kdrain, kdrain2))]
            vs = [step(700 if (w == 0 and s == 0) else 500, ("v", w, s),
                       lambda s=s: vstep(s))
                  for s in range(4)]
            # interleave: c-group deadlines are heads 2c, V(w) is needed by
            # the first P@V pop of window w. The alloc step (x DMA issue)
            # is returned separately -- the driver runs it immediately at
            # the previous window's start so the transfers land in time.
    

            return (step(0, ("qk", w, 0), alloc),
                    qk[0] + qk[1] + vs[0:1] + qk[2] + vs[1:2]
                    + qk[3] + vs[2:4])

        def emit_scores_exp(w, h, qt, feed):
            """DoubleRow fp8 scores + paired exp for head h of window w.
            Calls feed() after each pair (fill-step pacing). Returns the
            list of (exp_tile, sub) chunk handles."""
            kmax = 4 * (w + 1)
            ch, po = h // 2, (h % 2) * 64
            ex_buf = [None] * kmax
            jps = (1, 0) if w == 0 else range(kmax // 2)
            for jp in jps:
                if w == 0 and jp == 0:
                    force(("qk2", 0, ch))   # jp0 reads the patched tokens
                pssb = ps_s.tile([128, 2, 512], F32, tag="pss", name="pss")
                exb = exp_pool.tile([128, 2, 512], F16, tag="ex", name="ex")
                rel0 = 2 * jp - 4 * w
                # both matmuls write from the PAIR's first live column (the
                # second diag chunk's extra 128 columns are causally dead but
                # keep the paired exp's input region initialized)
                q0 = max(rel0, 0) * 128
                for sub in range(2):
                    j = 2 * jp + sub
                    corr = (w == 0 and j == 0) and ABL_CORR
                    nc.tensor.matmul(
                        pssb[:, sub, q0:],
                        lhsT=KT_t[po:po + 64, ch, :,
                                  128 * j:128 * (j + 1)].broadcast_to(
                                      [64, 2, 128]),
                        rhs=qt[po:po + 64, ch, :, q0:],
                        start=True, stop=not corr,
                        perf_mode=DR, skip_group_check=True,
                    )
                    if corr:
                        # += K_lo * (Q_hi + Q_lo) on the causally-peaked
                        # first square (k<128, q<128)
                        nc.tensor.matmul(
                            pssb[:, sub, 0:128],
                            lhsT=KLO_t[po:po + 64, ch, :, :].broadcast_to(
                                [64, 2, 128]),
                            rhs=qt[po:po + 64, ch, :, 0:128],
                            start=False, stop=True,
                            perf_mode=DR, skip_group_check=True,
                        )
                nc.scalar.activation(out=exb[:, :, q0:],
                                     in_=pssb[:, :, q0:],
                                     func=Exp, scale=0.125 / 1024.0)
                for sub in range(2):
                    rel = 2 * jp + sub - 4 * w
                    if rel >= 0:
                        qq = rel * 128
                        # zero exp(s) above the diagonal; only PV of
                        # sub-q i == rel reads this square
                        nc.gpsimd.tensor_mul(
                            exb[:, sub, qq:qq + 128],
                            exb[:, sub, qq:qq + 128], mask_t)
                ex_buf[2 * jp] = (exb, 0)
                ex_buf[2 * jp + 1] = (exb, 1)
                # pair's ACT time minus its PE time funds the fill budget
                feed((2 * (512 - q0)) * 0.833 + 185.0
                     - (2 * (512 - q0)) * 0.417)
            return ex_buf

        def emit_pv_sub(w, h, ex_buf, attn_t, i):
            """P@V' + rescale for one 128-query sub-chunk."""
            pso = ps_pv.tile([128, DH + 1], F32, tag="pso", name="pso")
            jlast = 4 * w + i
            for j in range(jlast + 1):
                exb, sub = ex_buf[j]
                nc.tensor.matmul(
                    pso,
                    lhsT=exb[:, sub, 128 * i:128 * (i + 1)],
                    rhs=Vp_t[:, j, h, :],
                    start=(j == 0), stop=(j == jlast),
                )
            rec = small.tile([128, 1], F32, tag="rec", name="rec")
            nc.vector.reciprocal(rec, pso[:, DH:DH + 1])
            nc.vector.tensor_mul(
                attn_t[:, i, DH * h:DH * (h + 1)],
                pso[:, 0:DH],
                rec.broadcast_to([128, DH]),
            )

        def emit_pv(w, h, ex_buf, attn_t):
            for i in range(4):
                emit_pv_sub(w, h, ex_buf, attn_t, i)

        def tail_sub(w, attn_t, i, last=False):
            """Transpose + W_O + store for one 128-query sub-chunk."""
            drain = nc.scalar.copy if last else nc.vector.tensor_copy
            atT = attnTp.tile([128, 4, 128], F16, tag="attnT", name="attnT")
            pst = ps_fill.tile([128, 512], F32, tag="fill", name="pst")
            for c in range(4):
                nc.tensor.transpose(
                    pst[:, 128 * c:128 * (c + 1)],
                    attn_t[:, i, 128 * c:128 * (c + 1)], ident_t)
            drain(atT, pst.rearrange("p (c q) -> p c q", c=4))
            ysb = ybuf.tile([128, 2, 512], F16, tag="ysb", name="ysb")
            for jc in range(2):
                py = ps_fill.tile([128, 512], F32, tag="fill", name="py")
                for c in range(4):
                    nc.tensor.matmul(
                        py,
                        lhsT=atT[:, c, :],
                        rhs=wo16_t[:, c, 512 * jc:512 * (jc + 1)],
                        start=(c == 0), stop=(c == 3),
                    )
                drain(ysb[:, jc, :], py)
            eng = nc.sync if last else nc.gpsimd
            eng.dma_start(
                out=y[512 * w + 128 * i:512 * w + 128 * (i + 1), :],
                in_=ysb,
            )

        # ---- driver ----
        pend = deque()    # (w, h, ex_buf, attn_t) awaiting P@V
        prio = deque()    # lazy P@V sub-chunk steps, drained before fills
        budget = [0.0]    # fill-step PE budget (ns), fed per score pair

        def feed(gain=611.0):
            budget[0] = min(budget[0] + gain, 1500.0)
            for q in (prio, fills):
                while q:
                    st = q[0]
                    if st["done"]:
                        q.popleft()
                        continue
                    if st["wt"] <= budget[0]:
                        budget[0] -= st["wt"]
                        q.popleft()
                        run_step(st)
                    else:
                        return

        def force_pv_all(pw):
            for ph in range(HLOC):
                force(("pv", pw, ph))

        def pv_sub_forced(pw, ph, pex, pat, i):
            for s in range(i + 1):
                force(("v", pw, s))
            emit_pv_sub(pw, ph, pex, pat, i)

        def enqueue_pv(pw, ph, pex, pat):
            """Queue P@V for (pw, ph) as 4 small prio steps (one per
            128-query sub-chunk) so feed() interleaves them into the
            ACT-paced score-pair stream."""
            if pw >= 2:
                force(("tail", pw - 2))   # attn ring (bufs=3) safety
            for i in range(4):
                prio.append(step(
                    (4 * pw + i + 1) * 27 + 60, ("pv", pw, ph),
                    lambda i=i: pv_sub_forced(pw, ph, pex, pat, i)))
            if ph == HLOC - 1:     # window pw fully rescaled -> tail it
                force(("wo",))
                for i in range(4):
                    def tfn(pw=pw, pat=pat, i=i):
                        force_pv_all(pw)
                        tail_sub(pw, pat, i)
                    fills.append(step(900, ("tail", pw), tfn))

        box0 = {}
        a0, s0 = proj_steps(0, box0)
        run_step(a0)
        force(("qk", 0, 0))        # Q/K c0: first scores input
        qt_cur = box0["qt"]
        fills.extend(s0)           # remaining w0 steps (done ones skipped)
        box = {}
        a1, nxt = proj_steps(1, box)
        run_step(a1)               # x8/x16(1) DMAs queue behind w0 loads
        fills.extend(nxt)
        wo_st = step(0, ("wo",),
                     lambda: nc.sync.dma_start(out=wo16_t, in_=wo16_r))
        run_step(wo_st)

        for w in range(TC):
            if 1 <= w < TC - 1:
                a2, s2 = proj_steps(w + 1, box)
                run_step(a2)       # issue x8/x16(w+1) DMAs now
                fills.extend(s2)
            if w >= 3:
                force(("tail", w - 3))   # attn ring slot reuse (bufs=3)
            attn_t = attnp.tile([128, 4, DSH], F32, tag="attn", name="attn_t")
            depth = PV_DEPTH[w]
            for h in range(HLOC):
                force(("qk", w, h // 2))
                last_head = (w == TC - 1 and h == HLOC - 1)
                if last_head:
                    # hand pending P@V to the prio queue so it overlaps
                    # this head's pairs
                    while pend:
                        pw, ph, pex, pat = pend.popleft()
                        enqueue_pv(pw, ph, pex, pat)
                ex = emit_scores_exp(w, h, qt_cur, feed)
                if last_head:
                    for st in list(prio):
                        run_step(st)
                    prio.clear()
                    for s in range(4):
                        force(("v", w, s))
                    force(("wo",))
                    # interleave PV+tail per sub-chunk; sub-q 2/3 both wait
                    # on the final exp pair, so emit their PVs together and
                    # pipeline the two tails
                    for i in range(2):
                        emit_pv_sub(w, h, ex, attn_t, i)
                        tail_sub(w, attn_t, i, last=True)
                    emit_pv_sub(w, h, ex, attn_t, 2)
                    emit_pv_sub(w, h, ex, attn_t, 3)
                    tail_sub(w, attn_t, 2, last=True)
                    tail_sub(w, attn_t, 3, last=True)
                    continue
                pend.append((w, h, ex, attn_t))
                npop = 0
                while len(pend) > depth and npop < 2:
                    npop += 1
                    pw, ph, pex, pat = pend.popleft()
                    enqueue_pv(pw, ph, pex, pat)
            if w + 1 < TC:
                force(("qk", w + 1, 0))  # next window's first scores input
                qt_cur = box["qt"]
                box = {}
        for st in list(prio) + list(fills):
            run_step(st)
    nc.compile()
    return nc


def shard_inputs(x, Wq, bq, Wk, bk, Wv, bv, Wo, bo):
    """Returns the 8 per-core input maps (host-side quantization)."""
    import ml_dtypes
    f8 = ml_dtypes.float8_e4m3fn
    # fp8e4m3's normal range floor is 2^-6: scale x by 4 and W by 8 so
    # weights (sigma 0.02) and their residuals escape the subnormal zone
    # while scaled scores (<=32k) stay inside fp16 range on the ACT input
    # path. The exp scale divides the 32^2 back out.
    XS, WS = 4.0, 8.0
    in_maps = []
    for c in range(N_CORES):
        b, g = c // 2, c % 2
        sl = slice(DSH * g, DSH * (g + 1))
        xT = np.ascontiguousarray(x[b].T)

        def pack_qk(W):
            a = W[sl].T.reshape(4, 2, 128, 4, 128)  # [jp, i, p, c, q]
            return (WS * np.ascontiguousarray(
                a.transpose(2, 3, 0, 1, 4))).astype(f8)

        x8_hi = (XS * xT).astype(f8)
        wv_a = np.ascontiguousarray(
            Wv[sl].T.reshape(4, 2, 128, DSH).transpose(2, 0, 1, 3))
        wv_hi = (WS * wv_a).astype(f8)
        wq_hi = pack_qk(Wq)
        wk_hi = pack_qk(Wk)

        def pack_qk_lo(W, hi):
            a = W[sl].T.reshape(4, 2, 128, 4, 128)
            a = np.ascontiguousarray(a.transpose(2, 3, 0, 1, 4))
            return (WS * a - hi.astype(np.float32)).astype(f8)

        in_maps.append({
            "x8": x8_hi,
            "x8lo": (XS * xT[:, 0:128]
                     - x8_hi[:, 0:128].astype(np.float32)).astype(f8),
            "wq8lo": pack_qk_lo(Wq, wq_hi),
            "wk8lo": pack_qk_lo(Wk, wk_hi),
            "wq8": wq_hi,
            "wk8": wk_hi,
            "wv8": wv_hi,
            "wv8lo": (WS * wv_a - wv_hi.astype(np.float32)).astype(f8),
            "wo16": np.ascontiguousarray(Wo.T[sl]).astype(np.float16),
            "bqp": (XS * WS * np.ascontiguousarray(bq[sl])).astype(np.float32),
            "bkp": (XS * WS * np.ascontiguousarray(bk[sl])).astype(np.float32),
        })
    return in_maps


def combine_outputs(results, bv, Wo, bo):
    """Sum head-group partials per batch + rank-1 bias corrections."""
    corr = (bv @ Wo.T + bo).astype(np.float32)  # [D]; exact because softmax
    y = np.empty((BATCH, T, D), dtype=np.float32)  # rows sum to 1
    for b in range(BATCH):
        y[b] = (results[2 * b]["y"].astype(np.float32)
                + results[2 * b + 1]["y"].astype(np.float32) + corr)
    return y


def run_sharded(inputs, trace=False):
    """Build, compile, run on cores 0-7. Returns (y_full, BassKernelResults)."""
    from concourse import bass_utils

    inputs = {k: np.asarray(v, dtype=np.float32) for k, v in inputs.items()}
    nc = _build()
    in_maps = shard_inputs(
        inputs["x"], inputs["Wq"], inputs["bq"], inputs["Wk"], inputs["bk"],
        inputs["Wv"], inputs["bv"], inputs["Wo"], inputs["bo"])
    res = bass_utils.run_bass_kernel_spmd(
        nc, in_maps, list(range(N_CORES)), trace=trace)
    y = combine_outputs(res.results, inputs["bv"], inputs["Wo"], inputs["bo"])
    return y, res


def kernel(**inputs):
    y, _ = run_sharded(inputs, trace=False)
    return y


if __name__ == "__main__":
    rng = np.random.default_rng(0)
    demo = {
        "x": rng.standard_normal((BATCH, T, D), dtype=np.float32),
        "Wq": rng.standard_normal((D, D), dtype=np.float32) * 0.02,
        "bq": np.zeros(D, np.float32),
        "Wk": rng.standard_normal((D, D), dtype=np.float32) * 0.02,
        "bk": np.zeros(D, np.float32),
        "Wv": rng.standard_normal((D, D), dtype=np.float32) * 0.02,
        "bv": np.zeros(D, np.float32),
        "Wo": rng.standard_normal((D, D), dtype=np.float32) * 0.02,
        "bo": np.zeros(D, np.float32),
    }
    out = kernel(**demo)
    print(out.shape, out.dtype)
